# revision 20
# baseline (speedup 1.0000x reference)
"""Trainium2 Bass kernel for the EighMSE loss (data-parallel over 8 cores).

Math (replicates jax/LAPACK ssyevd eigenvector sign conventions for 2x2
symmetric matrices):
  row (a, b, c) encodes [[a, b], [b, c]]
  SM = a + c, DF = a - c, CD = c - a = -DF, RT = sqrt(DF^2 + 4 b^2)
  closed-form evals = (SM +- RT) / 2
  x = clamp(DF / RT, -1, 1); n0 = sqrt((1+x)/2), n1 = sqrt((1-x)/2)
  LAPACK larger-eval eigenvector g = (tau0*n0, tau1*n1) with
    tau0 = -1 if DF > 0 else sign(b)*sign(SM);  tau1 = tau0*sign(b)
  smaller-eval eigenvector = sign(SM) * (-g1, g0)

Implementation (choices driven by the TRN2 cost model: DVE bf16
tensor_tensor = 2x, tensor_scalar = 4x; gpsimd(Pool) supports only
tensor_copy; ACT `abs_reciprocal_sqrt_and_small` table provides 1/sqrt):
  - pred and true are processed as one packed [P, 2W] bf16 stream
    (halves [0:W] = pred, [W:2W] = true); pair terms use the halves.
  - a, b, c f32->bf16 conversions run on the otherwise-idle Pool engine.
  - Signs handled bitwise on uint16 views: signbit(t2) = signbit(CD) OR
    (signbit(b) XOR signbit(SM)) == signbit(tau0); signbit(tau1) adds
    signbit(b); pair products get their sign via XOR of sign masks.
  - xh = clamp(0.5*CD/RT, +-(0.5-2^-9)) so n0 = sqrt(0.5 - xh),
    n1 = sqrt(0.5 + xh); sum n0^2 = 0.5*N - sum xh (E-expansion):
      E1 = N - Sx - 2*sum(Q0),  E2 = N + Sx - 2*sum(Q1),
      Q0 = g0p*g0t = (n0p*n0t) ^ (parity of tau0 signs), etc.
    W0/W1 = sign(SMp)sign(SMt)*Q0/Q1; G = sum_{sx} Q = (SQ - SW)/2.
  - Bs = sum dRT^2 = (S4b + SCD2) - 2*sum sqrt(RTsq_p*RTsq_t).
  - ACT ops grouped per tile: [Square/ARS table] then [Sqrt table].
  - The otherwise-idle PE engine does the xh/Q0/Q1/W0/W1 reductions as
    ones-vector matmuls accumulating in PSUM (f32), drained by DMA at
    the end and summed on host.

Per-core partial sums:
  stats (ACT accum, NACT per tile): 0: S4b = sum 4b^2 (both sides)
    1: SCD2 = sum CD^2 (both)  2: A = sum dSM^2  3: C = sum dCD^2
    4: Dm = sum db^2  5: SRTx = sum RTp*RTt
  psum (PE, [1, 512] per term): 0: Sx  1: SQ0  2: SQ1  3: SW0  4: SW1
"""

import numpy as np
from contextlib import ExitStack

import concourse.bass as bass
import concourse.bacc as bacc
import concourse.tile as tile
from concourse import mybir
from concourse.bass_utils import run_bass_kernel_spmd

F32 = mybir.dt.float32
BF16 = mybir.dt.bfloat16
U16 = mybir.dt.uint16
OP = mybir.AluOpType
AF = mybir.ActivationFunctionType

B_TOTAL = 4_194_304
NCORES = 8
S = B_TOTAL // NCORES          # samples per core
P = 128                        # partitions
NPC = S // P                   # samples per partition (4096)
W = 1024                       # max samples per tile per partition
WS = [256, 768, 1024, 1024, 1024]   # per-iteration widths (sum = NPC)
NT = len(WS)
NACT = 6                       # ACT-accumulated stats per tile
NPE = 7                        # PE terms (Sx, SQ0, SQ1, SW0, SW1, SCDx, Sbx)
CH = 512                       # matmul rhs chunk (max free size)
XCLIP = 0.498046875            # 0.5 - 2^-9, bf16-exact; keeps sqrt args > 0

_BUILT = None
TRACE = False
LAST_RESULT = None


def _build_bass():
    nc = bacc.Bacc()
    yp = nc.declare_dram_parameter("y_pred", [S, 3], F32, isOutput=False)
    yt = nc.declare_dram_parameter("y_true", [S, 3], F32, isOutput=False)
    out = nc.declare_dram_parameter("out", [P, NACT], F32, isOutput=True)
    out2 = nc.declare_dram_parameter("out2", [1, NPE], F32, isOutput=True)

    ypr = yp.rearrange("(p n) c -> p n c", p=P)
    ytr = yt.rearrange("(p n) c -> p n c", p=P)

    def xor_(o, a, b):
        nc.vector.tensor_tensor(
            o.bitcast(U16), a.bitcast(U16), b.bitcast(U16), op=OP.bitwise_xor
        )

    def or_(o, a, b):
        nc.vector.tensor_tensor(
            o.bitcast(U16), a.bitcast(U16), b.bitcast(U16), op=OP.bitwise_or
        )

    def andm(o, a):
        nc.vector.tensor_scalar(
            o.bitcast(U16), a.bitcast(U16), 0x8000, None, op0=OP.bitwise_and
        )

    with tile.TileContext(nc) as tc, ExitStack() as ctx:
        inp = ctx.enter_context(tc.tile_pool(name="inp", bufs=2))
        cvt = ctx.enter_context(tc.tile_pool(name="cvt", bufs=2))
        wk = ctx.enter_context(tc.tile_pool(name="wk", bufs=2))
        wkp = ctx.enter_context(tc.tile_pool(name="wkp", bufs=2))
        accp = ctx.enter_context(tc.tile_pool(name="accp", bufs=1))
        psp = ctx.enter_context(tc.tile_pool(name="psp", bufs=1, space="PSUM"))

        stats = accp.tile([P, NT * NACT], F32)
        nc.gpsimd.memset(stats[:], 0.0)
        halfc = accp.tile([P, 1], F32)
        nc.vector.memset(halfc[:], 0.5)
        ones = accp.tile([P, 1], BF16)
        nc.vector.memset(ones[:], 1.0)
        scrA = accp.tile([P, W], F32)   # ACT accumulate scratch

        ps = []
        for k in range(NPE):
            pst = psp.tile([1, CH], F32, tag=f"ps{k}", name=f"ps{k}")
            ps.append(pst)

        def pe_sum(src2d, k, first, last):
            # src2d: [P, nw] bf16 SBUF; accumulate column sums into ps[k]
            nw = src2d.shape[-1]
            nch = -(-nw // CH)
            for jj, j in enumerate(range(0, nw, CH)):
                c = min(CH, nw - j)
                nc.tensor.matmul(
                    ps[k][:, :c], ones[:],
                    src2d[:, j : j + c],
                    start=(first and jj == 0), stop=(last and jj == nch - 1),
                    skip_group_check=True,
                )

        PAIRS = [[0, 1], [2, 3], [4]]
        offs = [sum(WS[:i]) for i in range(NT)]
        st = [{} for _ in range(NT)]

        def emit_load(it):
            Wi, woff, sd = WS[it], offs[it], st[it]
            xp = inp.tile([P, W, 3], F32, tag="xp")
            nc.sync.dma_start(xp[:, :Wi, :], ypr[:, woff : woff + Wi, :])
            xt = inp.tile([P, W, 3], F32, tag="xt")
            nc.sync.dma_start(xt[:, :Wi, :], ytr[:, woff : woff + Wi, :])
            aBf = cvt.tile([P, 2 * W], BF16, tag="aB")
            bBf = cvt.tile([P, 2 * W], BF16, tag="bB")
            cBf = cvt.tile([P, 2 * W], BF16, tag="cB")
            for h, x in ((0, xp), (1, xt)):
                nc.gpsimd.tensor_copy(aBf[:, h * Wi : (h + 1) * Wi], x[:, :Wi, 0])
                nc.gpsimd.tensor_copy(cBf[:, h * Wi : (h + 1) * Wi], x[:, :Wi, 2])
            for h, x in ((0, xp), (1, xt)):
                nc.gpsimd.tensor_copy(bBf[:, h * Wi : (h + 1) * Wi], x[:, :Wi, 1])
            sd.update(aB=aBf[:, : 2 * Wi], bB=bBf[:, : 2 * Wi], cB=cBf[:, : 2 * Wi])

        def emit_chainB(it):
            Wi, sd = WS[it], st[it]
            first, last = it == 0, it == NT - 1
            col = lambda k: stats[:, it * NACT + k : it * NACT + k + 1]
            aB, bB, cB = sd["aB"], sd["bB"], sd["cB"]
            SMf = wk.tile([P, 2 * W], BF16, tag="SM")
            SM = SMf[:, : 2 * Wi]
            nc.vector.tensor_add(SM, aB, cB)
            CDf = wk.tile([P, 2 * W], BF16, tag="CD")
            CD = CDf[:, : 2 * Wi]
            nc.vector.tensor_sub(CD, cB, aB)
            b4sqf = wk.tile([P, 2 * W], BF16, tag="b4sq")
            b4sq = b4sqf[:, : 2 * Wi]
            nc.scalar.activation(b4sq, bB, AF.Square, scale=2.0, accum_out=col(0))
            CDsqf = wk.tile([P, 2 * W], BF16, tag="CDsq")
            CDsq = CDsqf[:, : 2 * Wi]
            nc.scalar.activation(CDsq, CD, AF.Square, accum_out=col(1))
            RTsqf = wk.tile([P, 2 * W], BF16, tag="RTsq")
            RTsq = RTsqf[:, : 2 * Wi]
            nc.vector.tensor_add(RTsq, b4sq, CDsq)
            rsf = wk.tile([P, 2 * W], BF16, tag="rs")
            rs = rsf[:, : 2 * Wi]
            nc.scalar.activation(rs, RTsq, AF.Abs_reciprocal_sqrt)
            xhf = wk.tile([P, 2 * W], BF16, tag="xh")
            xh = xhf[:, : 2 * Wi]
            nc.vector.tensor_mul(xh, CD, rs)
            nc.vector.tensor_scalar(xh, xh, 0.5, XCLIP, op0=OP.mult, op1=OP.min)
            nc.vector.tensor_scalar(xh, xh, -XCLIP, None, op0=OP.max)
            pe_sum(xh, 0, first, last)
            t2f = wk.tile([P, 2 * W], BF16, tag="t2")
            t2 = t2f[:, : 2 * Wi]
            xor_(t2, bB, SM)
            or_(t2, t2, CD)
            sd.update(SM=SM, CD=CD, RTsq=RTsq, xh=xh, t2=t2)

        def emit_chainA(it):
            Wi, sd = WS[it], st[it]
            n0f = wk.tile([P, 2 * W], BF16, tag="n0")
            n0 = n0f[:, : 2 * Wi]
            nc.scalar.activation(n0, sd["xh"], AF.Sqrt, scale=-1.0, bias=halfc[:])
            n1f = wk.tile([P, 2 * W], BF16, tag="n1")
            n1 = n1f[:, : 2 * Wi]
            nc.scalar.activation(n1, sd["xh"], AF.Sqrt, scale=1.0, bias=halfc[:])
            sd.update(n0=n0, n1=n1)

        def emit_pair_terms(it):
            Wi, sd = WS[it], st[it]
            first, last = it == 0, it == NT - 1
            col = lambda k: stats[:, it * NACT + k : it * NACT + k + 1]
            SM, CD, bB, RTsq, t2 = sd["SM"], sd["CD"], sd["bB"], sd["RTsq"], sd["t2"]
            n0, n1 = sd["n0"], sd["n1"]
            hp = slice(0, Wi)
            ht = slice(Wi, 2 * Wi)
            Q0f = wkp.tile([P, W], BF16, tag="Q0")
            Q0 = Q0f[:, :Wi]
            nc.vector.tensor_mul(Q0, n0[:, hp], n0[:, ht])
            Q1f = wkp.tile([P, W], BF16, tag="Q1")
            Q1 = Q1f[:, :Wi]
            nc.vector.tensor_mul(Q1, n1[:, hp], n1[:, ht])
            e0f = wkp.tile([P, W], BF16, tag="e0")
            e0 = e0f[:, :Wi]
            xor_(e0, t2[:, hp], t2[:, ht])
            vf = wkp.tile([P, W], BF16, tag="v")
            v = vf[:, :Wi]
            xor_(v, bB[:, hp], bB[:, ht])
            xor_(v, e0, v)
            andm(e0, e0)
            andm(v, v)
            xor_(Q0, Q0, e0)
            pe_sum(Q0, 1, first, last)
            xor_(Q1, Q1, v)
            pe_sum(Q1, 2, first, last)
            wf = wkp.tile([P, W], BF16, tag="w")
            w = wf[:, :Wi]
            xor_(w, SM[:, hp], SM[:, ht])
            andm(w, w)
            W0f = wkp.tile([P, W], BF16, tag="W0")
            W0 = W0f[:, :Wi]
            xor_(W0, Q0, w)
            pe_sum(W0, 3, first, last)
            W1f = wkp.tile([P, W], BF16, tag="W1")
            W1 = W1f[:, :Wi]
            xor_(W1, Q1, w)
            pe_sum(W1, 4, first, last)
            d0f = wkp.tile([P, W], BF16, tag="d0")
            d0 = d0f[:, :Wi]
            nc.vector.tensor_sub(d0, SM[:, hp], SM[:, ht])
            nc.scalar.activation(scrA[:, :Wi], d0, AF.Square, accum_out=col(2))
            d1f = wkp.tile([P, W], BF16, tag="d1")
            d1 = d1f[:, :Wi]
            nc.vector.tensor_mul(d1, CD[:, hp], CD[:, ht])    # CDp*CDt
            pe_sum(d1, 5, first, last)
            d2f = wkp.tile([P, W], BF16, tag="d2")
            d2 = d2f[:, :Wi]
            nc.vector.tensor_mul(d2, bB[:, hp], bB[:, ht])    # bp*bt
            pe_sum(d2, 6, first, last)
            pif = wkp.tile([P, W], BF16, tag="pi")
            pi = pif[:, :Wi]
            nc.vector.tensor_mul(pi, RTsq[:, hp], RTsq[:, ht])
            nc.scalar.activation(scrA[:, :Wi], pi, AF.Sqrt, accum_out=col(5))

        for pair in PAIRS:
            for it in pair:
                emit_load(it)
            for it in pair:
                emit_chainB(it)
            for it in pair:
                emit_chainA(it)
            for it in pair:
                emit_pair_terms(it)

        outsums = accp.tile([P, NACT], F32)
        stats3 = stats[:].rearrange("p (t k) -> p k t", t=NT)
        for k in range(NACT):
            nc.vector.tensor_reduce(
                outsums[:, k : k + 1], stats3[:, k, :],
                axis=mybir.AxisListType.X, op=OP.add,
            )
        nc.sync.dma_start(out[:, :], outsums[:])
        pesums = accp.tile([1, NPE], F32)
        for k in range(NPE):
            nc.vector.tensor_reduce(
                pesums[:, k : k + 1], ps[k][:, :],
                axis=mybir.AxisListType.X, op=OP.add,
            )
        nc.sync.dma_start(out2[:, :], pesums[:])

    nc.compile()
    return nc


def _get_built():
    global _BUILT
    if _BUILT is None:
        _BUILT = _build_bass()
    return _BUILT


def kernel(y_pred: np.ndarray, y_true: np.ndarray, weights: np.ndarray) -> np.ndarray:
    global LAST_RESULT
    y_pred = np.ascontiguousarray(y_pred, dtype=np.float32)
    y_true = np.ascontiguousarray(y_true, dtype=np.float32)
    w = np.asarray(weights, dtype=np.float64)

    nc = _get_built()
    in_maps = []
    for c in range(NCORES):
        in_maps.append(
            {
                "y_pred": y_pred[c * S : (c + 1) * S],
                "y_true": y_true[c * S : (c + 1) * S],
            }
        )
    res = run_bass_kernel_spmd(nc, in_maps, list(range(NCORES)), trace=TRACE)
    LAST_RESULT = res
    sums = np.zeros(NACT, dtype=np.float64)
    pes = np.zeros(NPE, dtype=np.float64)
    for c in range(NCORES):
        sums += np.asarray(res.results[c]["out"], dtype=np.float64).sum(axis=0)
        pes += np.asarray(res.results[c]["out2"], dtype=np.float64).reshape(-1)

    S4b, SCD2, A, _, _, SRTx = sums
    Sx, SQ0, SQ1, SW0, SW1, SCDx, Sbx = pes
    C = SCD2 - 2.0 * SCDx
    Dm = 0.25 * S4b - 2.0 * Sbx
    N = float(B_TOTAL)
    E1 = N - Sx - 2.0 * SQ0
    E2 = N + Sx - 2.0 * SQ1
    G0 = 0.5 * (SQ0 - SW0)
    G1 = 0.5 * (SQ1 - SW1)
    F0 = E1 + 4.0 * G0
    F1 = E2 + 4.0 * G1
    Bs = (S4b + SCD2) - 2.0 * SRTx      # sum dRT^2
    evals_mse = (A + Bs) / (4.0 * N)
    mse_loss = (0.5 * A + 0.5 * C + Dm) / (3.0 * N)
    loss = (
        w[0] * evals_mse
        + w[1] * E1 / N
        + w[2] * E2 / N
        + w[3] * F1 / N
        + w[4] * F0 / N
        + w[5] * mse_loss
    )
    return np.float32(loss)


# revision 21
# speedup vs baseline: 1.0348x; 1.0348x over previous
"""Trainium2 Bass kernel for the EighMSE loss (data-parallel over 8 cores).

Math (replicates jax/LAPACK ssyevd eigenvector sign conventions for 2x2
symmetric matrices):
  row (a, b, c) encodes [[a, b], [b, c]]
  SM = a + c, DF = a - c, CD = c - a = -DF, RT = sqrt(DF^2 + 4 b^2)
  closed-form evals = (SM +- RT) / 2
  x = clamp(DF / RT, -1, 1); n0 = sqrt((1+x)/2), n1 = sqrt((1-x)/2)
  LAPACK larger-eval eigenvector g = (tau0*n0, tau1*n1) with
    tau0 = -1 if DF > 0 else sign(b)*sign(SM);  tau1 = tau0*sign(b)
  smaller-eval eigenvector = sign(SM) * (-g1, g0)

Implementation (choices driven by the TRN2 cost model: DVE bf16
tensor_tensor = 2x, tensor_scalar = 4x; gpsimd(Pool) supports only
tensor_copy; ACT `abs_reciprocal_sqrt_and_small` table provides 1/sqrt):
  - pred and true are processed as one packed [P, 2W] bf16 stream
    (halves [0:W] = pred, [W:2W] = true); pair terms use the halves.
  - a, b, c f32->bf16 conversions run on the otherwise-idle Pool engine.
  - Signs handled bitwise on uint16 views: signbit(t2) = signbit(CD) OR
    (signbit(b) XOR signbit(SM)) == signbit(tau0); signbit(tau1) adds
    signbit(b); pair products get their sign via XOR of sign masks.
  - xh = clamp(0.5*CD/RT, +-(0.5-2^-9)) so n0 = sqrt(0.5 - xh),
    n1 = sqrt(0.5 + xh); sum n0^2 = 0.5*N - sum xh (E-expansion):
      E1 = N - Sx - 2*sum(Q0),  E2 = N + Sx - 2*sum(Q1),
      Q0 = g0p*g0t = (n0p*n0t) ^ (parity of tau0 signs), etc.
    W0/W1 = sign(SMp)sign(SMt)*Q0/Q1; G = sum_{sx} Q = (SQ - SW)/2.
  - Bs = sum dRT^2 = (S4b + SCD2) - 2*sum sqrt(RTsq_p*RTsq_t).
  - ACT ops grouped per tile: [Square/ARS table] then [Sqrt table].
  - The otherwise-idle PE engine does the xh/Q0/Q1/W0/W1 reductions as
    ones-vector matmuls accumulating in PSUM (f32), drained by DMA at
    the end and summed on host.

Per-core partial sums:
  stats (ACT accum, NACT per tile): 0: S4b = sum 4b^2 (both sides)
    1: SCD2 = sum CD^2 (both)  2: A = sum dSM^2  3: C = sum dCD^2
    4: Dm = sum db^2  5: SRTx = sum RTp*RTt
  psum (PE, [1, 512] per term): 0: Sx  1: SQ0  2: SQ1  3: SW0  4: SW1
"""

import numpy as np
from contextlib import ExitStack

import concourse.bass as bass
import concourse.bacc as bacc
import concourse.tile as tile
from concourse import mybir
from concourse.bass_utils import run_bass_kernel_spmd

F32 = mybir.dt.float32
BF16 = mybir.dt.bfloat16
U16 = mybir.dt.uint16
OP = mybir.AluOpType
AF = mybir.ActivationFunctionType

B_TOTAL = 4_194_304
NCORES = 8
S = B_TOTAL // NCORES          # samples per core
P = 128                        # partitions
NPC = S // P                   # samples per partition (4096)
W = 1024                       # max samples per tile per partition
WS = [256, 768, 1024, 1024, 1024]   # per-iteration widths (sum = NPC)
NT = len(WS)
NACT = 6                       # ACT-accumulated stats per tile
NPE = 7                        # PE terms (Sx, SQ0, SQ1, SW0, SW1, SCDx, Sbx)
CH = 512                       # matmul rhs chunk (max free size)
XCLIP = 0.498046875            # 0.5 - 2^-9, bf16-exact; keeps sqrt args > 0

_BUILT = None
TRACE = False
LAST_RESULT = None


def _build_bass():
    nc = bacc.Bacc()
    yp = nc.declare_dram_parameter("y_pred", [S, 3], F32, isOutput=False)
    yt = nc.declare_dram_parameter("y_true", [S, 3], F32, isOutput=False)
    out = nc.declare_dram_parameter("out", [P, NACT], F32, isOutput=True)
    out2 = nc.declare_dram_parameter("out2", [1, NPE], F32, isOutput=True)

    ypr = yp.rearrange("(p n) c -> p n c", p=P)
    ytr = yt.rearrange("(p n) c -> p n c", p=P)

    def xor_(o, a, b):
        nc.vector.tensor_tensor(
            o.bitcast(U16), a.bitcast(U16), b.bitcast(U16), op=OP.bitwise_xor
        )

    def or_(o, a, b):
        nc.vector.tensor_tensor(
            o.bitcast(U16), a.bitcast(U16), b.bitcast(U16), op=OP.bitwise_or
        )

    def andm(o, a):
        nc.vector.tensor_scalar(
            o.bitcast(U16), a.bitcast(U16), 0x8000, None, op0=OP.bitwise_and
        )

    with tile.TileContext(nc) as tc, ExitStack() as ctx:
        inp = ctx.enter_context(tc.tile_pool(name="inp", bufs=2))
        cvt = ctx.enter_context(tc.tile_pool(name="cvt", bufs=2))
        wk = ctx.enter_context(tc.tile_pool(name="wk", bufs=2))
        wkp = ctx.enter_context(tc.tile_pool(name="wkp", bufs=2))
        accp = ctx.enter_context(tc.tile_pool(name="accp", bufs=1))
        psp = ctx.enter_context(tc.tile_pool(name="psp", bufs=1, space="PSUM"))

        stats = accp.tile([P, NT * NACT], F32)
        nc.gpsimd.memset(stats[:], 0.0)
        halfc = accp.tile([P, 1], F32)
        nc.vector.memset(halfc[:], 0.5)
        ones = accp.tile([P, 1], BF16)
        nc.vector.memset(ones[:], 1.0)
        scrA = accp.tile([P, W], F32)   # ACT accumulate scratch

        ps = []
        for k in range(NPE):
            pst = psp.tile([1, CH], F32, tag=f"ps{k}", name=f"ps{k}")
            ps.append(pst)

        def pe_sum(src2d, k, first, last):
            # src2d: [P, nw] bf16 SBUF; accumulate column sums into ps[k]
            nw = src2d.shape[-1]
            nch = -(-nw // CH)
            for jj, j in enumerate(range(0, nw, CH)):
                c = min(CH, nw - j)
                nc.tensor.matmul(
                    ps[k][:, :c], ones[:],
                    src2d[:, j : j + c],
                    start=(first and jj == 0), stop=(last and jj == nch - 1),
                    skip_group_check=True,
                )

        PAIRS = [[0, 1], [2, 3], [4]]
        offs = [sum(WS[:i]) for i in range(NT)]
        st = [{} for _ in range(NT)]

        def emit_load(it):
            Wi, woff, sd = WS[it], offs[it], st[it]
            xp = inp.tile([P, W, 3], F32, tag="xp")
            nc.sync.dma_start(xp[:, :Wi, :], ypr[:, woff : woff + Wi, :])
            xt = inp.tile([P, W, 3], F32, tag="xt")
            nc.sync.dma_start(xt[:, :Wi, :], ytr[:, woff : woff + Wi, :])
            aBf = cvt.tile([P, 2 * W], BF16, tag="aB")
            bBf = cvt.tile([P, 2 * W], BF16, tag="bB")
            cBf = cvt.tile([P, 2 * W], BF16, tag="cB")
            for h, x in ((0, xp), (1, xt)):
                nc.gpsimd.tensor_copy(aBf[:, h * Wi : (h + 1) * Wi], x[:, :Wi, 0])
                nc.gpsimd.tensor_copy(cBf[:, h * Wi : (h + 1) * Wi], x[:, :Wi, 2])
            for h, x in ((0, xp), (1, xt)):
                nc.gpsimd.tensor_copy(bBf[:, h * Wi : (h + 1) * Wi], x[:, :Wi, 1])
            sd.update(aB=aBf[:, : 2 * Wi], bB=bBf[:, : 2 * Wi], cB=cBf[:, : 2 * Wi])

        def emit_chainB(it):
            Wi, sd = WS[it], st[it]
            first, last = it == 0, it == NT - 1
            col = lambda k: stats[:, it * NACT + k : it * NACT + k + 1]
            aB, bB, cB = sd["aB"], sd["bB"], sd["cB"]
            SMf = wk.tile([P, 2 * W], BF16, tag="SM")
            SM = SMf[:, : 2 * Wi]
            nc.vector.tensor_add(SM, aB, cB)
            CDf = wk.tile([P, 2 * W], BF16, tag="CD")
            CD = CDf[:, : 2 * Wi]
            nc.vector.tensor_sub(CD, cB, aB)
            b4sqf = wk.tile([P, 2 * W], BF16, tag="b4sq")
            b4sq = b4sqf[:, : 2 * Wi]
            nc.scalar.activation(b4sq, bB, AF.Square, scale=2.0, accum_out=col(0))
            CDsqf = wk.tile([P, 2 * W], BF16, tag="CDsq")
            CDsq = CDsqf[:, : 2 * Wi]
            nc.scalar.activation(CDsq, CD, AF.Square, accum_out=col(1))
            RTsqf = wk.tile([P, 2 * W], BF16, tag="RTsq")
            RTsq = RTsqf[:, : 2 * Wi]
            nc.vector.tensor_add(RTsq, b4sq, CDsq)
            rsf = wk.tile([P, 2 * W], BF16, tag="rs")
            rs = rsf[:, : 2 * Wi]
            nc.scalar.activation(rs, RTsq, AF.Abs_reciprocal_sqrt, scale=4.0)
            xhf = wk.tile([P, 2 * W], BF16, tag="xh")
            xh = xhf[:, : 2 * Wi]
            nc.vector.tensor_mul(xh, CD, rs)
            nc.vector.tensor_scalar(xh, xh, XCLIP, -XCLIP, op0=OP.min, op1=OP.max)
            pe_sum(xh, 0, first, last)
            t2f = wk.tile([P, 2 * W], BF16, tag="t2")
            t2 = t2f[:, : 2 * Wi]
            xor_(t2, bB, SM)
            or_(t2, t2, CD)
            sd.update(SM=SM, CD=CD, RTsq=RTsq, xh=xh, t2=t2)

        def emit_chainA(it):
            Wi, sd = WS[it], st[it]
            n0f = wk.tile([P, 2 * W], BF16, tag="n0")
            n0 = n0f[:, : 2 * Wi]
            nc.scalar.activation(n0, sd["xh"], AF.Sqrt, scale=-1.0, bias=halfc[:])
            n1f = wk.tile([P, 2 * W], BF16, tag="n1")
            n1 = n1f[:, : 2 * Wi]
            nc.scalar.activation(n1, sd["xh"], AF.Sqrt, scale=1.0, bias=halfc[:])
            sd.update(n0=n0, n1=n1)

        def emit_pair_terms(it):
            Wi, sd = WS[it], st[it]
            first, last = it == 0, it == NT - 1
            col = lambda k: stats[:, it * NACT + k : it * NACT + k + 1]
            SM, CD, bB, RTsq, t2 = sd["SM"], sd["CD"], sd["bB"], sd["RTsq"], sd["t2"]
            n0, n1 = sd["n0"], sd["n1"]
            hp = slice(0, Wi)
            ht = slice(Wi, 2 * Wi)
            Q0f = wkp.tile([P, W], BF16, tag="Q0")
            Q0 = Q0f[:, :Wi]
            nc.vector.tensor_mul(Q0, n0[:, hp], n0[:, ht])
            Q1f = wkp.tile([P, W], BF16, tag="Q1")
            Q1 = Q1f[:, :Wi]
            nc.vector.tensor_mul(Q1, n1[:, hp], n1[:, ht])
            Ef = wkp.tile([P, 3 * W], BF16, tag="E")
            e0 = Ef[:, 0 * Wi : 1 * Wi]
            e1 = Ef[:, 1 * Wi : 2 * Wi]
            w = Ef[:, 2 * Wi : 3 * Wi]
            xor_(e0, t2[:, hp], t2[:, ht])
            xor_(e1, bB[:, hp], bB[:, ht])
            xor_(e1, e0, e1)
            xor_(w, SM[:, hp], SM[:, ht])
            Eall = Ef[:, : 3 * Wi]
            andm(Eall, Eall)
            xor_(Q0, Q0, e0)
            pe_sum(Q0, 1, first, last)
            xor_(Q1, Q1, e1)
            pe_sum(Q1, 2, first, last)
            W0f = wkp.tile([P, W], BF16, tag="W0")
            W0 = W0f[:, :Wi]
            xor_(W0, Q0, w)
            pe_sum(W0, 3, first, last)
            W1f = wkp.tile([P, W], BF16, tag="W1")
            W1 = W1f[:, :Wi]
            xor_(W1, Q1, w)
            pe_sum(W1, 4, first, last)
            d0f = wkp.tile([P, W], BF16, tag="d0")
            d0 = d0f[:, :Wi]
            nc.vector.tensor_sub(d0, SM[:, hp], SM[:, ht])
            nc.scalar.activation(scrA[:, :Wi], d0, AF.Square, accum_out=col(2))
            d1f = wkp.tile([P, W], BF16, tag="d1")
            d1 = d1f[:, :Wi]
            nc.vector.tensor_mul(d1, CD[:, hp], CD[:, ht])    # CDp*CDt
            pe_sum(d1, 5, first, last)
            d2f = wkp.tile([P, W], BF16, tag="d2")
            d2 = d2f[:, :Wi]
            nc.vector.tensor_mul(d2, bB[:, hp], bB[:, ht])    # bp*bt
            pe_sum(d2, 6, first, last)
            pif = wkp.tile([P, W], BF16, tag="pi")
            pi = pif[:, :Wi]
            nc.vector.tensor_mul(pi, RTsq[:, hp], RTsq[:, ht])
            nc.scalar.activation(scrA[:, :Wi], pi, AF.Sqrt, accum_out=col(5))

        for pair in PAIRS:
            for it in pair:
                emit_load(it)
            for it in pair:
                emit_chainB(it)
            for it in pair:
                emit_chainA(it)
            for it in pair:
                emit_pair_terms(it)

        outsums = accp.tile([P, NACT], F32)
        stats3 = stats[:].rearrange("p (t k) -> p k t", t=NT)
        for k in range(NACT):
            nc.vector.tensor_reduce(
                outsums[:, k : k + 1], stats3[:, k, :],
                axis=mybir.AxisListType.X, op=OP.add,
            )
        nc.sync.dma_start(out[:, :], outsums[:])
        pesums = accp.tile([1, NPE], F32)
        for k in range(NPE):
            nc.vector.tensor_reduce(
                pesums[:, k : k + 1], ps[k][:, :],
                axis=mybir.AxisListType.X, op=OP.add,
            )
        nc.sync.dma_start(out2[:, :], pesums[:])

    nc.compile()
    return nc


def _get_built():
    global _BUILT
    if _BUILT is None:
        _BUILT = _build_bass()
    return _BUILT


def kernel(y_pred: np.ndarray, y_true: np.ndarray, weights: np.ndarray) -> np.ndarray:
    global LAST_RESULT
    y_pred = np.ascontiguousarray(y_pred, dtype=np.float32)
    y_true = np.ascontiguousarray(y_true, dtype=np.float32)
    w = np.asarray(weights, dtype=np.float64)

    nc = _get_built()
    in_maps = []
    for c in range(NCORES):
        in_maps.append(
            {
                "y_pred": y_pred[c * S : (c + 1) * S],
                "y_true": y_true[c * S : (c + 1) * S],
            }
        )
    res = run_bass_kernel_spmd(nc, in_maps, list(range(NCORES)), trace=TRACE)
    LAST_RESULT = res
    sums = np.zeros(NACT, dtype=np.float64)
    pes = np.zeros(NPE, dtype=np.float64)
    for c in range(NCORES):
        sums += np.asarray(res.results[c]["out"], dtype=np.float64).sum(axis=0)
        pes += np.asarray(res.results[c]["out2"], dtype=np.float64).reshape(-1)

    S4b, SCD2, A, _, _, SRTx = sums
    Sx, SQ0, SQ1, SW0, SW1, SCDx, Sbx = pes
    C = SCD2 - 2.0 * SCDx
    Dm = 0.25 * S4b - 2.0 * Sbx
    N = float(B_TOTAL)
    E1 = N - Sx - 2.0 * SQ0
    E2 = N + Sx - 2.0 * SQ1
    G0 = 0.5 * (SQ0 - SW0)
    G1 = 0.5 * (SQ1 - SW1)
    F0 = E1 + 4.0 * G0
    F1 = E2 + 4.0 * G1
    Bs = (S4b + SCD2) - 2.0 * SRTx      # sum dRT^2
    evals_mse = (A + Bs) / (4.0 * N)
    mse_loss = (0.5 * A + 0.5 * C + Dm) / (3.0 * N)
    loss = (
        w[0] * evals_mse
        + w[1] * E1 / N
        + w[2] * E2 / N
        + w[3] * F1 / N
        + w[4] * F0 / N
        + w[5] * mse_loss
    )
    return np.float32(loss)
